# revision 11
# baseline (speedup 1.0000x reference)
"""Trainium2 Bass kernel for nn_Bilinear_23115513987566 (PGA geometric-algebra
bilinear layer). Self-contained: builds all constant tables at import time.

Math: out = equi_linear(concat(gp(lg,rg), ref_mv14 * join(lj,rj)), W_out)
where [lg|rg|lj|rj] = equi_linear(x, W_bil).

Factorization (validated to ~1e-16 in numpy):
  gp: e0-grading split + Cl(3,0)=Mat2(C) rep + Gauss 3-mult complex products
      -> 72 elementwise products per channel, all pre/post maps constant.
  jn: dualize -> Grassmann wedge in Lambda(R^4) -> 81 products per channel,
      signs folded into constant expansion matrices.
Constant maps are folded into PE matmul weights; products run on DVE.

Sharding: pure data parallel over 8 cores (8192 tokens each), no comms.
"""
import math
import os
from contextlib import ExitStack

import numpy as np

# =========================================================================
# Constant tables (PGA G(3,0,1))
# =========================================================================
METRIC = np.array([0.0, 1.0, 1.0, 1.0])
BLADES = [(), (0,), (1,), (2,), (3,), (0, 1), (0, 2), (0, 3), (1, 2), (1, 3),
          (2, 3), (0, 1, 2), (0, 1, 3), (0, 2, 3), (1, 2, 3), (0, 1, 2, 3)]
IDX = {b: i for i, b in enumerate(BLADES)}
NCH = 16


def _sort_sign(lst):
    lst = list(lst)
    sign = 1
    for i in range(len(lst)):
        for j in range(len(lst) - 1 - i):
            if lst[j] > lst[j + 1]:
                lst[j], lst[j + 1] = lst[j + 1], lst[j]
                sign = -sign
    return lst, sign


def _outer(a, b):
    if set(a) & set(b):
        return (), 0.0
    lst, sign = _sort_sign(list(a) + list(b))
    return tuple(lst), float(sign)


def _build_tables():
    O = np.zeros((16, 16, 16))
    for i, a in enumerate(BLADES):
        for j, b in enumerate(BLADES):
            bl, c = _outer(a, b)
            if c != 0.0:
                O[IDX[bl], i, j] = c
    D = np.zeros((16, 16))
    for i, a in enumerate(BLADES):
        comp = tuple(sorted(set(range(4)) - set(a)))
        _, s = _outer(a, comp)
        D[IDX[comp], i] = s
    Dinv = np.linalg.inv(D)
    B = np.zeros((9, 16, 16))
    grades = [len(b) for b in BLADES]
    for w in range(5):
        for i, g in enumerate(grades):
            if g == w:
                B[w, i, i] = 1.0
    for w in range(4):
        for i, a in enumerate(BLADES):
            if len(a) == w and 0 not in a:
                bl, c = _outer((0,), a)
                B[5 + w, IDX[bl], i] = c
    return B, D, Dinv


EQUI_BASIS, DUAL, DUALINV = _build_tables()

# Pauli rep of Cl(3,0): T_MU [8 mu-coords (p,q,reim), 8 blades m3]
_I2 = np.eye(2, dtype=complex)
_PAULI = {1: np.array([[0, 1], [1, 0]], dtype=complex),
          2: np.array([[0, -1j], [1j, 0]], dtype=complex),
          3: np.array([[1, 0], [0, -1]], dtype=complex)}


def _rho(mask3):
    M = _I2.copy()
    for g in (1, 2, 3):
        if mask3 >> (g - 1) & 1:
            M = M @ _PAULI[g]
    return M


T_MU = np.zeros((8, 8))
for _m3 in range(8):
    _M = _rho(_m3)
    for _p in range(2):
        for _q in range(2):
            T_MU[_p * 4 + _q * 2 + 0, _m3] = _M[_p, _q].real
            T_MU[_p * 4 + _q * 2 + 1, _m3] = _M[_p, _q].imag
T_MU_INV = np.linalg.inv(T_MU)
INVOL3 = np.array([(-1.0) ** bin(m).count("1") for m in range(8)])

REF_OF = {}
for _i, _b in enumerate(BLADES):
    _e0 = 1 if 0 in _b else 0
    _m3 = sum(1 << (g - 1) for g in _b if g != 0)
    REF_OF[(_e0, _m3)] = _i
MASK_OF_REF = np.array([sum(1 << g for g in b) for b in BLADES])
REF_OF_MASK = np.zeros(16, dtype=int)
for _i, _m in enumerate(MASK_OF_REF):
    REF_OF_MASK[_m] = _i

# quads of ref-blade indices, AP-affine: (base, [(step,2),(step,2)])
QUADS = [(0, [(14, 2), (1, 2)]), (2, [(1, 2), (3, 2)]),
         (4, [(4, 2), (3, 2)]), (9, [(1, 2), (3, 2)])]
QUAD_REFS = [[b + i1 * d[0][0] + i2 * d[1][0] for i1 in range(2) for i2 in range(2)]
             for b, d in QUADS]
XROW_FEAT = np.zeros(512, dtype=int)
for _j in range(4):
    for _m in range(4):
        for _c in range(32):
            XROW_FEAT[_j * 128 + _m * 32 + _c] = _c * 16 + QUAD_REFS[_j][_m]

# ---- gp feature maps (tiles over h = [64ch x 16 refblades] = 1024) ----
A_GPL2 = np.zeros((6, 96, 1024))
A_GPR2 = np.zeros((4, 96, 1024))
for lmat in range(3):
    _e0 = 1 if lmat == 2 else 0
    _sgn = INVOL3 if lmat == 1 else np.ones(8)
    for dL in range(2):
        for r in range(2):
            p = r ^ dL
            re_row, im_row = T_MU[p * 4 + r * 2], T_MU[p * 4 + r * 2 + 1]
            for c in range(NCH):
                for m3 in range(8):
                    a = REF_OF[(_e0, m3)]
                    rr = r * 48 + c * 3
                    h = c * 16 + a
                    A_GPL2[lmat * 2 + dL, rr + 0, h] += re_row[m3] * _sgn[m3]
                    A_GPL2[lmat * 2 + dL, rr + 1, h] += im_row[m3] * _sgn[m3]
                    A_GPL2[lmat * 2 + dL, rr + 2, h] += (re_row[m3] + im_row[m3]) * _sgn[m3]
for rmat in range(2):
    for dR in range(2):
        for r in range(2):
            q = r ^ dR
            re_row, im_row = T_MU[r * 4 + q * 2], T_MU[r * 4 + q * 2 + 1]
            for c in range(NCH):
                for m3 in range(8):
                    a = REF_OF[(rmat, m3)]
                    rr = r * 48 + c * 3
                    h = (16 + c) * 16 + a
                    A_GPR2[rmat * 2 + dR, rr + 0, h] += re_row[m3]
                    A_GPR2[rmat * 2 + dR, rr + 1, h] += im_row[m3]
                    A_GPR2[rmat * 2 + dR, rr + 2, h] += re_row[m3] + im_row[m3]

# ---- jn dist tiles ----
A_JNL2 = np.zeros((2, 128, 1024))
A_JNR2 = np.zeros((2, 128, 1024))
for m4 in range(16):
    li = REF_OF_MASK[m4]
    for j in range(16):
        if DUAL[li, j] != 0.0:
            for c in range(NCH):
                A_JNL2[m4 // 8, (m4 % 8) * NCH + c, (32 + c) * 16 + j] += DUAL[li, j]
                A_JNR2[m4 // 8, (m4 % 8) * NCH + c, (48 + c) * 16 + j] += DUAL[li, j]


def _sgn4(A, B):
    ta = tuple(g for g in range(4) if A >> g & 1)
    tb = tuple(g for g in range(4) if B >> g & 1)
    _, s = _sort_sign(list(ta) + list(tb))
    return float(s)


JN_PROD = []
for S in range(16):
    subs = sorted((A for A in range(16) if (A & ~S & 15) == 0),
                  key=lambda A: (A // 8, bin(A).count("1"), A))
    for A in subs:
        JN_PROD.append((S, A, S ^ A))
JN_NM = len(JN_PROD) * NCH          # 1296
JN_NCHUNK = (JN_NM + 127) // 128    # 11

EXPL = np.zeros((JN_NM, 2, 128))
EXPR = np.zeros((JN_NM, 2, 128))
for mi, (S, A, B) in enumerate(JN_PROD):
    for c in range(NCH):
        EXPL[mi * NCH + c, A // 8, (A % 8) * NCH + c] = 1.0
        EXPR[mi * NCH + c, B // 8, (B % 8) * NCH + c] = _sgn4(A, B)

# ---- V maps ----
V_GP2 = np.zeros((2, 128, 12, 96))
for t in range(3):
    part = 0 if t == 0 else 1
    for dL in range(2):
        for dR in range(2):
            tile = (t * 2 + dL) * 2 + dR
            for r in range(2):
                p, q = r ^ dL, r ^ dR
                for c in range(NCH):
                    rr = r * 48 + c * 3
                    re_i = (p * 4 + q * 2 + 0) * NCH + c
                    im_i = (p * 4 + q * 2 + 1) * NCH + c
                    V_GP2[part, re_i, tile, rr + 0] += 1.0
                    V_GP2[part, re_i, tile, rr + 1] += -1.0
                    V_GP2[part, im_i, tile, rr + 2] += 1.0
                    V_GP2[part, im_i, tile, rr + 0] += -1.0
                    V_GP2[part, im_i, tile, rr + 1] += -1.0
V_JN2 = np.zeros((2, 128, JN_NM))
for mi, (S, A, B) in enumerate(JN_PROD):
    for c in range(NCH):
        V_JN2[S // 8, (S % 8) * NCH + c, mi * NCH + c] += 1.0

# ---- back maps into stage2 ----
B_GP2 = np.zeros((256, 256))
for c in range(NCH):
    for m3 in range(8):
        for part in range(2):
            a = REF_OF[(part, m3)]
            for mu in range(8):
                B_GP2[c * 16 + a, part * 128 + mu * NCH + c] += T_MU_INV[m3, mu]
B_JN2 = np.zeros((256, 256))
for c in range(NCH):
    for S in range(16):
        wref = REF_OF_MASK[S]
        for k in range(16):
            if DUALINV[k, wref] != 0.0:
                B_JN2[c * 16 + k, (S // 8) * 128 + (S % 8) * NCH + c] += DUALINV[k, wref]


def _build_M1(W_bil):
    M = np.einsum('ocw,wab->oacb', W_bil, EQUI_BASIS)
    return M.reshape(64 * 16, 32 * 16)


def _build_W2p(W_out):
    M2 = np.einsum('ocw,wab->oacb', W_out, EQUI_BASIS).reshape(512, 512)
    BY = np.zeros((512, 512))
    BY[:256, :256] = B_GP2
    BY[256:, 256:] = B_JN2
    return M2 @ BY


def build_schedule(W_bil, W_out):
    M1 = _build_M1(W_bil)
    W2p = _build_W2p(W_out)
    cols = []
    ncols = [0]

    def add_block(blk):
        K, M = blk.shape
        b = np.zeros((128, M), dtype=np.float32)
        b[:K] = blk
        cols.append(b)
        ofs = ncols[0]
        ncols[0] += M
        return ofs

    sched = {"equi1": [], "exp": [], "vmap": [], "st2": []}

    def equi1_tile(name, Amat):
        W = Amat @ M1
        Wp = W[:, XROW_FEAT]
        for j in range(4):
            blk = Wp[:, j * 128:(j + 1) * 128]
            if np.any(blk != 0.0):
                sched["equi1"].append((name, j, add_block(blk.T.copy()), blk.shape[0]))

    for t in range(6):
        equi1_tile(("gpl", t), A_GPL2[t])
    for t in range(4):
        equi1_tile(("gpr", t), A_GPR2[t])
    for t in range(2):
        equi1_tile(("jnl", t), A_JNL2[t])
    for t in range(2):
        equi1_tile(("jnr", t), A_JNR2[t])

    for side, EXP in (("L", EXPL), ("R", EXPR)):
        for ci in range(JN_NCHUNK):
            lo, hi = ci * 128, min((ci + 1) * 128, JN_NM)
            for dt in range(2):
                blk = EXP[lo:hi, dt, :]
                if np.any(blk != 0.0):
                    sched["exp"].append((side, ci, dt, add_block(blk.T.copy()), hi - lo))

    for part in range(2):
        for tile in range(12):
            blk = V_GP2[part][:, tile, :]
            if np.any(blk != 0.0):
                sched["vmap"].append((part, "mgp", tile, add_block(blk.T.copy()), 128, 96))
    for part in range(2):
        for ci in range(JN_NCHUNK):
            lo, hi = ci * 128, min((ci + 1) * 128, JN_NM)
            blk = V_JN2[part][:, lo:hi]
            if np.any(blk != 0.0):
                sched["vmap"].append((2 + part, "mjn", ci, add_block(blk.T.copy()), 128, hi - lo))

    for oc in range(4):
        for yc in range(4):
            blk = W2p[oc * 128:(oc + 1) * 128, yc * 128:(yc + 1) * 128]
            if np.any(blk != 0.0):
                sched["st2"].append((oc, yc, add_block(blk.T.copy()), 128))

    wcat = np.concatenate(cols, axis=1).astype(np.float32)
    return wcat, sched


# =========================================================================
# Bass program
# =========================================================================
N_CORES = 8
TOK_PER_CORE = 8192
T = 512                       # tokens per pipeline tile
NTILE = TOK_PER_CORE // T
MM_DT_NAME = os.environ.get("KB_MM_DT", "float32r")

_PROG_CACHE = {}


def _build_program(nw_cols, sched):
    import concourse.bacc as bacc
    import concourse.bass as bass
    import concourse.tile as tile
    from concourse import mybir
    from concourse.masks import make_identity

    f32 = mybir.dt.float32
    mm_dt = getattr(mybir.dt, MM_DT_NAME)

    nc = bacc.Bacc("TRN2", target_bir_lowering=False, debug=False)
    x_d = nc.dram_tensor("x", [TOK_PER_CORE, 512], f32, kind="ExternalInput")
    rm_d = nc.dram_tensor("refmv", [TOK_PER_CORE, 16], f32, kind="ExternalInput")
    w_d = nc.dram_tensor("wcat", [128, nw_cols], mm_dt, kind="ExternalInput")
    y_d = nc.dram_tensor("y", [TOK_PER_CORE, 512], f32, kind="ExternalOutput")

    # group schedules
    equi1_by_dst = {}
    for name, j, wofs, M in sched["equi1"]:
        equi1_by_dst.setdefault(name, []).append((j, wofs, M))
    exp_by_dst = {}
    for side, ci, dt_, wofs, M in sched["exp"]:
        exp_by_dst.setdefault((side, ci), []).append((dt_, wofs, M))
    vmap_by_dst = {}
    for yc, kind, idx, wofs, M, K in sched["vmap"]:
        vmap_by_dst.setdefault(yc, []).append((kind, idx, wofs, M, K))
    st2_by_dst = {}
    for oc, yc, wofs, M in sched["st2"]:
        st2_by_dst.setdefault(oc, []).append((yc, wofs, M))

    LMAT = [0, 1, 2]
    RMAT = [0, 1, 0]

    with ExitStack() as ctx:
        tc = ctx.enter_context(tile.TileContext(nc))
        const = ctx.enter_context(tc.tile_pool(name="const", bufs=1))
        xin = ctx.enter_context(tc.tile_pool(name="xin", bufs=2))
        rmin = ctx.enter_context(tc.tile_pool(name="rmin", bufs=2))
        xtp = ctx.enter_context(tc.tile_pool(name="xtp", bufs=5))
        hgp = ctx.enter_context(tc.tile_pool(name="hgp", bufs=7))
        hjn = ctx.enter_context(tc.tile_pool(name="hjn", bufs=5))
        expp = ctx.enter_context(tc.tile_pool(name="expp", bufs=2))
        mgp = ctx.enter_context(tc.tile_pool(name="mgp", bufs=3))
        mjn = ctx.enter_context(tc.tile_pool(name="mjn", bufs=3))
        ysb = ctx.enter_context(tc.tile_pool(name="ysb", bufs=5))
        osb = ctx.enter_context(tc.tile_pool(name="osb", bufs=5))
        outp = ctx.enter_context(tc.tile_pool(name="outp", bufs=2))
        smal = ctx.enter_context(tc.tile_pool(name="smal", bufs=2))
        psum = ctx.enter_context(tc.tile_pool(name="psum", bufs=8, space="PSUM"))

        w_sb = const.tile([128, nw_cols], mm_dt)
        nc.sync.dma_start(out=w_sb[:], in_=w_d.ap())
        ident = const.tile([128, 128], f32)
        make_identity(nc, ident[:])

        def W(wofs, M, K=128):
            return w_sb[:K, wofs:wofs + M]

        for ti in range(NTILE):
            tok = slice(ti * T, (ti + 1) * T)
            xt = xin.tile([128, 4, 512], f32)
            nc.sync.dma_start(out=xt[:], in_=x_d.ap()[tok, :].rearrange("(s p) f -> p s f", s=4))
            rmt = rmin.tile([128, 4, 16], f32)
            nc.sync.dma_start(out=rmt[:], in_=rm_d.ap()[tok, :].rearrange("(s p) f -> p s f", s=4))

            # ---- permute x features into quad-grouped order (POOL copies) ----
            xq = xin.tile([128, 4, 512], f32, tag="xq")
            for j, (base, dims) in enumerate(QUADS):
                in_ap = bass.AP(
                    tensor=xt.tensor,
                    offset=xt.offset + base,
                    ap=[[2048, 128], [512, 4], [dims[0][0], 2], [dims[1][0], 2],
                        [16, 32]],
                )
                out_ap = bass.AP(
                    tensor=xq.tensor,
                    offset=xq.offset + j * 128,
                    ap=[[2048, 128], [512, 4], [64, 2], [32, 2], [1, 32]],
                )
                nc.gpsimd.tensor_copy(out=out_ap, in_=in_ap)

            # ---- x transposes ----
            xTs = []
            for j in range(4):
                ps = psum.tile([128, T], f32, tag="ps")
                for s in range(4):
                    nc.tensor.transpose(ps[:, s * 128:(s + 1) * 128],
                                        xq[:, s, j * 128:(j + 1) * 128], ident[:])
                xT = xtp.tile([128, T], mm_dt, tag="xT")
                nc.scalar.copy(out=xT[:], in_=ps[:])
                xTs.append(xT)

            # ---- refmv row -> broadcast tile ----
            psr = psum.tile([1, T], f32, tag="ps")
            for s in range(4):
                nc.tensor.transpose(psr[:, s * 128:(s + 1) * 128], rmt[:, s, 14:15], ident[:])
            rrow = smal.tile([1, T], f32, tag="rrow")
            nc.scalar.copy(out=rrow[:], in_=psr[:])
            rbc = smal.tile([128, T], f32, tag="rbc")
            nc.gpsimd.partition_broadcast(rbc[:], rrow[:])

            def run_equi1(name, nrows):
                entries = equi1_by_dst[name]
                ps = psum.tile([nrows, T], f32, tag="ps")
                for i, (j, wofs, M) in enumerate(entries):
                    nc.tensor.matmul(ps[:], W(wofs, M), xTs[j][:],
                                     start=(i == 0), stop=(i == len(entries) - 1))
                return ps

            # V-stage psum accumulators (streamed: V matmul right after each
            # product so m-tiles are short-lived)
            y_ps = []
            for _yi in range(4):
                ytile = psum.tile([128, T], f32, tag="ps", name=f"yps{_yi}")
                y_ps.append(ytile)
            v_count = {yc: 0 for yc in range(4)}
            v_total = {yc: len(vmap_by_dst[yc]) for yc in range(4)}
            # index V entries by source tile
            v_by_src = {}
            for yc, entries in vmap_by_dst.items():
                for kind, idx, wofs, M, K in entries:
                    v_by_src.setdefault((kind, idx), []).append((yc, wofs, M, K))

            def emit_v(kind, idx, rhs):
                for yc, wofs, M, K in v_by_src[(kind, idx)]:
                    i = v_count[yc]
                    nc.tensor.matmul(y_ps[yc][:], W(wofs, M, K), rhs[:],
                                     start=(i == 0), stop=(i == v_total[yc] - 1))
                    v_count[yc] += 1

            # ---- gp: equi1 L tiles -> SBUF; R tiles stay PSUM, streamed ----
            gpl_sb = []
            for t in range(6):
                ps = run_equi1(("gpl", t), 96)
                sb = hgp.tile([96, T], f32, tag="gpl")
                nc.vector.tensor_copy(out=sb[:], in_=ps[:])
                gpl_sb.append(sb)
            for dR in range(2):
                for rmat in range(2):
                    psr_t = run_equi1(("gpr", rmat * 2 + dR), 96)
                    for t in range(3):
                        if RMAT[t] != rmat:
                            continue
                        for dL in range(2):
                            mt = mgp.tile([96, T], mm_dt, tag="mgp")
                            nc.vector.tensor_mul(
                                out=mt[:], in0=gpl_sb[LMAT[t] * 2 + dL][:], in1=psr_t[:])
                            emit_v("mgp", (t * 2 + dL) * 2 + dR, mt)

            # ---- jn dist ----
            jnl_sb = []
            for t in range(2):
                ps = run_equi1(("jnl", t), 128)
                sb = hjn.tile([128, T], mm_dt, tag="jnd")
                nc.scalar.copy(out=sb[:], in_=ps[:])
                jnl_sb.append(sb)
            jnr_sb = []
            for t in range(2):
                ps = run_equi1(("jnr", t), 128)
                sb = hjn.tile([128, T], mm_dt, tag="jnd")
                nc.vector.tensor_mul(out=sb[:], in0=ps[:], in1=rbc[:])
                jnr_sb.append(sb)

            # ---- jn expansion + products + streamed V ----
            for ci in range(JN_NCHUNK):
                nrows = min(128, JN_NM - ci * 128)
                entL = exp_by_dst[("L", ci)]
                psl = psum.tile([nrows, T], f32, tag="ps")
                for i, (dt_, wofs, M) in enumerate(entL):
                    nc.tensor.matmul(psl[:], W(wofs, M), jnl_sb[dt_][:],
                                     start=(i == 0), stop=(i == len(entL) - 1))
                lexp = expp.tile([nrows, T], f32, tag="lexp")
                nc.scalar.copy(out=lexp[:], in_=psl[:])
                entR = exp_by_dst[("R", ci)]
                psr2 = psum.tile([nrows, T], f32, tag="ps")
                for i, (dt_, wofs, M) in enumerate(entR):
                    nc.tensor.matmul(psr2[:], W(wofs, M), jnr_sb[dt_][:],
                                     start=(i == 0), stop=(i == len(entR) - 1))
                mt = mjn.tile([nrows, T], mm_dt, tag="mjn")
                nc.vector.tensor_mul(out=mt[:], in0=lexp[:], in1=psr2[:])
                emit_v("mjn", ci, mt)

            # ---- evac y' ----
            y_sb = []
            for yc in range(4):
                sb = ysb.tile([128, T], mm_dt, tag="ysb")
                nc.scalar.copy(out=sb[:], in_=y_ps[yc][:])
                y_sb.append(sb)

            # ---- stage2 ----
            o_sb = []
            for oc in range(4):
                entries = st2_by_dst[oc]
                ps = psum.tile([128, T], f32, tag="ps")
                for i, (yc, wofs, M) in enumerate(entries):
                    nc.tensor.matmul(ps[:], W(wofs, M), y_sb[yc][:],
                                     start=(i == 0), stop=(i == len(entries) - 1))
                sb = osb.tile([128, T], f32, tag="osb")
                nc.scalar.copy(out=sb[:], in_=ps[:])
                o_sb.append(sb)

            # ---- out transposes -> token-major -> DMA ----
            ot = outp.tile([128, 4, 512], f32)
            for s in range(4):
                ps = psum.tile([128, 512], f32, tag="ps")
                for fc in range(4):
                    nc.tensor.transpose(ps[:, fc * 128:(fc + 1) * 128],
                                        o_sb[fc][:, s * 128:(s + 1) * 128], ident[:])
                nc.scalar.copy(out=ot[:, s, :], in_=ps[:])
            nc.sync.dma_start(out=y_d.ap()[tok, :].rearrange("(s p) f -> p s f", s=4),
                              in_=ot[:])

    nc.finalize()
    return nc


def _get_program(nw_cols, sched_key, sched):
    key = (nw_cols, sched_key, MM_DT_NAME)
    if key not in _PROG_CACHE:
        _PROG_CACHE[key] = _build_program(nw_cols, sched)
    return _PROG_CACHE[key]


def kernel(x, ref_mv, W_bil, W_out):
    from concourse.bass_utils import run_bass_kernel_spmd

    x = np.ascontiguousarray(np.asarray(x), dtype=np.float32)
    ref_mv = np.ascontiguousarray(np.asarray(ref_mv), dtype=np.float32)
    W_bil = np.asarray(W_bil, dtype=np.float64)
    W_out = np.asarray(W_out, dtype=np.float64)

    Bn, S = x.shape[0], x.shape[1]
    ntok = Bn * S
    assert ntok == N_CORES * TOK_PER_CORE, (Bn, S)
    xf = x.reshape(ntok, 512)
    rf = ref_mv.reshape(ntok, 16)

    wcat, sched = build_schedule(W_bil, W_out)
    sched_key = tuple((k, len(v)) for k, v in sched.items())
    nc = _get_program(wcat.shape[1], sched_key, sched)

    in_maps = []
    for i in range(N_CORES):
        sl = slice(i * TOK_PER_CORE, (i + 1) * TOK_PER_CORE)
        in_maps.append({"x": xf[sl], "refmv": rf[sl], "wcat": wcat})
    res = run_bass_kernel_spmd(nc, in_maps, core_ids=list(range(N_CORES)))
    out = np.concatenate([r["y"] for r in res.results], axis=0)
    return out.reshape(Bn, S, 32, 16).astype(np.float32)
